# revision 43
# baseline (speedup 1.0000x reference)
"""Triplet-margin loss (EuclideanTriple) on 8 Trainium2 NeuronCores.

loss = sum_i relu( ||x_i - y_i + eps||_2 + margin - ||x_i - z_i + eps||_2 )

Data-parallel: N=131072 rows sharded 8 ways (16384 rows/core, no
collectives). Host sums the 8 cores' 32 per-partition partials.

v3 design (vs the f32 row-major baseline at ~167-181 us; measured
~63-64 us/pass):
1. Inputs are downcast to TRN fp8 e4m3 on the host (max +-240, data <6;
   loss rel err 1.0e-4 vs gate 2e-2) -> HBM reads drop 4x to 12.6
   MB/core. The SWDGE cast-DMA path (nc.gpsimd.dma_start with fp8 dram
   src, bf16 SBUF dst) upcasts in flight, so compute keeps bf16 perf
   modes. The wall is now the SBUF-write side of that DMA: 25.2 MB at
   ~430 GB/s (fabric-saturated) = ~58.6 us floor; plain-bf16-in-HBM was
   HBM-read-bound at 73-75 us.
2. TRANSPOSED layout: D=256 lives on partitions (two 128-halves), rows
   on the free dim. This moves the per-row sum over D from DVE
   tensor_reduce (1x mode, was ~60us/pass) to the otherwise-idle PE:
   matmul with a one-hot [128,32] stationary contracts the partition
   dim and drops each 512-row group's sums into its own PSUM partition
   (the zero rows accumulate harmlessly; start=True only on the first
   matmul per bank per pass).
3. Engine balance (measured spans/pass): DVE 16 bf16 subs (2x TT) ~44us
   + 1/8 of the squares as tensor_mul (dve_sq=1); ACT squares run at 1x
   rate REGARDLESS of dtype ((N+352)/1.2ns — the scalar engine has no
   bf16 packing), so 7/8 of squares on ACT ~50us; PE 128 matmuls adds
   ~3us overlapped. Timing loop: For_i unroll=8 + PE branch hints
   amortizes the ~2us back-edge barrier and the end-of-pass drain chain
   (worth ~16us/pass vs unroll=1).

Per-chunk dataflow (chunk = 2048 rows):
  DMA  : one cast-load of t3[128, 6*2048] bf16 from 1.5 MB fp8 dram
         (per-partition span x_h0|x_h1|y_h0|y_h1|z_h0|z_h1, contiguous)
  DVE  : ut = x - y, vt = x - z     (bf16 TT, 2x mode, FD=4096)
  ACT  : ut = (ut + eps)^2 in place (Square, eps rides the bias;
         last dve_sq 512-slices done as DVE tensor_mul, eps dropped
         there: |2*eps*u| <= 2.4e-5/elem)
  PE   : per 512-row group g: 2 accum matmuls (D-halves) with one-hot
         stationary col g -> psum_dp[g, :512] (+ same for vt -> psum_dn)
Tail per pass: ACT sqrt psum->sbuf, DVE hinge sub, ACT Relu(+margin
bias) with accum_out -> [32,1], DMA out. A dummy preamble Sqrt pins the
sqrt_and_others ACT table set (contains Square+Sqrt+Relu) so the loop
body has no LoadActFuncSet.

Measured dead ends: gpsimd upcast of raw-fp8 chunks (gp_up) serializes
the SWDGE queue, 2x slower; raw-fp8 subs at DVE 1x (raw8) lose more than
the DMA-write saving; staggered_reset no-op; uv_bufs=3 slightly worse.
"""

from contextlib import ExitStack

import numpy as np
import ml_dtypes

import concourse.bacc as bacc
import concourse.bass as bass
import concourse.mybir as mybir
import concourse.tile as tile
from concourse import bass_utils

N_TOTAL = 131072
D = 256
N_CORES = 8
SHARD = N_TOTAL // N_CORES   # 16384 rows per core
P = 128                      # SBUF partitions; D = 2*P halves
CHUNK_N = 2048               # rows per chunk
N_CHUNKS = SHARD // CHUNK_N  # 8
GROUP = 512                  # rows per psum group (one bank column span)
GPC = CHUNK_N // GROUP       # 4 groups per chunk
N_GROUPS = SHARD // GROUP    # 32 groups -> psum partitions 0..31
MARGIN = 0.5
EPS = 1e-6
F32 = mybir.dt.float32
BF16 = mybir.dt.bfloat16
IO_BUFS = 3
UV_BUFS = 2
# Best measured config: fp8 inputs (cast to bf16 during DMA), fine
# per-(tensor, D-half) sub/square splits (finer deps -> earlier ACT/PE
# release; coarse fused tiles measured 8us WORSE), 2 of 8 square-slices
# per tensor offloaded to DVE, 4 io bufs; timing loop uses For_i
# unroll 16 with PE branch hints.
BEST_IN_DT = "fp8"
BEST_KW = {
    "in_dt": BEST_IN_DT,
    "hints": True,
    "unroll": 16,
    "fine": True,
    "dve_sq": 2,
    "io_bufs": 4,
    "dsplit": True,
}


def build_nc(
    repeat: int = 1,
    mode: str = "full",
    io_bufs: int = IO_BUFS,
    uv_bufs: int = UV_BUFS,
    loop: bool = False,
    unroll: int = 1,
    fq: str = "s",
    dve_sq: int = 0,
    chunk_n: int = CHUNK_N,
    stagger: bool = False,
    hints: bool = False,
    in_dt: str = "bf16",
    gp_up: int = 0,
    raw8: int = 0,
    fuse_uv: bool = False,
    fine: bool = False,
    dsplit: bool = False,
    ps_bufs: int = 2,
) -> bass.Bass:
    """mode: 'full' | 'dma' (loads only) | 'compute' (no loads) |
    'nosq' (subs only) | 'nored' (subs+squares, no PE/tail).
    dve_sq: number of 512-col slices per chunk (0..8) whose square runs
    on DVE (tensor_mul, no eps) instead of ACT, to rebalance engines."""
    # dsplit: h-major DRAM layout (x_h0|y_h0|z_h0|x_h1|y_h1|z_h1) + two
    # half-chunk cast-DMAs, so the h0 sub/square/matmul chain starts when
    # half the chunk has landed (subtile deps). Requires the fine path.
    if dsplit:
        assert fine, "dsplit requires fine=True"
    cn = chunk_n
    n_chunks = SHARD // cn
    gpc = cn // GROUP
    act = mybir.ActivationFunctionType
    nc = bacc.Bacc("TRN2", target_bir_lowering=False, debug=False)

    # fp8 input: HBM holds float8e4 (TRN e4m3, max +-240; our data <6),
    # the DMA upcasts to bf16 on the way into SBUF (SWDGE cast path), so
    # all compute keeps bf16 perf modes while HBM reads halve again.
    dram_dt = BF16 if in_dt == "bf16" else mybir.dt.float8e4
    xyz = nc.dram_tensor(
        "xyz", [n_chunks, P, 6 * cn], dram_dt, kind="ExternalInput"
    ).ap()
    out = nc.dram_tensor("out", [N_GROUPS, 1], F32, kind="ExternalOutput").ap()

    with tile.TileContext(nc) as tc:
        with ExitStack() as ctx:
            io = ctx.enter_context(tc.tile_pool(name="io", bufs=io_bufs))
            uv = ctx.enter_context(tc.tile_pool(name="uv", bufs=uv_bufs))
            if gp_up or raw8:
                # fp8 staging tiles for chunks that come in raw over HWDGE
                # (cuts the SBUF-write-side DMA bytes, which bound the
                # cast-DMA floor)
                io8 = ctx.enter_context(tc.tile_pool(name="io8", bufs=2))
            # dp/dn each get ps_bufs banks; pass k's tail sqrt (queued on
            # the busy ACT FIFO) releases banks late, so extra bufs keep
            # the next passes' start=True matmuls from stalling on PSUM WAR
            ps = ctx.enter_context(
                tc.tile_pool(name="ps", bufs=ps_bufs, space="PSUM")
            )
            acc = ctx.enter_context(tc.tile_pool(name="acc", bufs=1))
            outp = ctx.enter_context(tc.tile_pool(name="outp", bufs=2))

            # persistent constants
            eps_t = acc.tile([P, 1], F32, tag="eps")
            nc.vector.memset(eps_t[:], EPS)
            mar_t = acc.tile([N_GROUPS, 1], F32, tag="mar")
            nc.vector.memset(mar_t[:], MARGIN)
            # Dummy Sqrt so the resident-set fixpoint sees sqrt_and_others
            # (which also contains Square and Relu) loaded on every path
            # into the loop body -> no ~2.7us LoadActFuncSet per pass.
            warm_t = acc.tile([P, 1], F32, tag="warm")
            nc.scalar.activation(warm_t[:], eps_t[:], mybir.ActivationFunctionType.Sqrt)
            # 32 one-hot stationaries: block g = sta[:, g*32:(g+1)*32] is
            # [128, 32] with column g all-ones. matmul(dp, block_g, mv)
            # adds mv's partition-sums into psum partition g (zeros into
            # the other 31 rows, harmless under accumulate).
            sta = acc.tile([P, N_GROUPS * 32], BF16, tag="sta")
            nc.vector.memset(sta[:], 0.0)
            for g in range(N_GROUPS):
                nc.vector.memset(sta[:, g * 33 : g * 33 + 1], 1.0)

            if mode == "compute":
                for _ in range(io_bufs):
                    t = io.tile([P, 6 * cn], BF16, tag="xyzt")
                    nc.vector.memset(t[:], 0.0)

            feng = {
                "s": nc.sync,
                "a": nc.scalar,
                "v": nc.vector,
                "p": nc.gpsimd,
                "t": nc.tensor,
            }

            def rep_body():
                dp = ps.tile([N_GROUPS, GROUP], F32, tag="dp")
                dn = ps.tile([N_GROUPS, GROUP], F32, tag="dn")
                nodma = mode == "compute" or mode.startswith("c_")
                # spread the gpsimd-upcast / raw-fp8 chunks evenly
                def spread(m):
                    return {
                        (i * n_chunks) // m + (n_chunks // m) // 2
                        for i in range(m)
                    } if m else set()
                up_set = spread(gp_up)
                raw_set = spread(raw8)
                for k in range(n_chunks):
                    src = None
                    if not nodma and in_dt == "fp8" and k in raw_set:
                        # raw fp8 load (HWDGE, 1 byte/elem on the SBUF write
                        # side); the subs below read fp8 at DVE 1x instead
                        src = io8.tile([P, 6 * cn], mybir.dt.float8e4,
                                       tag="t8")
                        nc.sync.dma_start(src[:], xyz[k])
                    else:
                        t3 = io.tile([P, 6 * cn], BF16, tag="xyzt")
                        src = t3
                        if not nodma:
                            if in_dt == "fp8" and k in up_set:
                                t8 = io8.tile([P, 6 * cn], mybir.dt.float8e4,
                                              tag="t8")
                                nc.sync.dma_start(t8[:], xyz[k])
                                nc.gpsimd.tensor_copy(t3[:], t8[:])
                            elif in_dt == "fp8" and dsplit:
                                nc.gpsimd.dma_start(
                                    t3[:, : 3 * cn], xyz[k][:, : 3 * cn]
                                )
                                nc.gpsimd.dma_start(
                                    t3[:, 3 * cn :], xyz[k][:, 3 * cn :]
                                )
                            elif in_dt == "fp8":
                                nc.gpsimd.dma_start(t3[:], xyz[k])  # SWDGE
                            else:
                                feng[fq[k % len(fq)]].dma_start(t3[:], xyz[k])
                        else:
                            # cheap writer so the scheduler sees the tile as
                            # allocated; contents are stale-but-defined
                            nc.vector.memset(t3[:, 0:1], 0.0)
                    if mode == "dma":
                        continue
                    xt = src[:, 0 : 2 * cn]
                    yt = src[:, 2 * cn : 4 * cn]
                    zt = src[:, 4 * cn : 6 * cn]
                    if fuse_uv:
                        uvt = uv.tile([P, 4 * cn], BF16, tag="uvt")
                        ut = uvt[:, : 2 * cn]
                        vt = uvt[:, 2 * cn :]
                    else:
                        ut = uv.tile([P, 2 * cn], BF16, tag="ut")
                        vt = uv.tile([P, 2 * cn], BF16, tag="vt")
                    if fine:
                        # per-(tensor, D-half) subs: each square (and its
                        # PE matmuls) can start after half the DVE work.
                        # dsplit: h-major source offsets, h-outer order so
                        # the h0 pair only waits on the first half-DMA
                        def sub_src(t_i, h):
                            if dsplit:
                                base = 3 * cn * h
                                return base, base + (t_i + 1) * cn
                            return h * cn, 2 * cn * (t_i + 1) + h * cn
                        for h in range(2):
                            for t_i, dst in ((0, ut), (1, vt)):
                                a_off, b_off = sub_src(t_i, h)
                                nc.vector.tensor_sub(
                                    dst[:, h * cn : (h + 1) * cn],
                                    src[:, a_off : a_off + cn],
                                    src[:, b_off : b_off + cn],
                                )
                    else:
                        nc.vector.tensor_sub(ut[:], xt, yt)
                        nc.vector.tensor_sub(vt[:], xt, zt)
                    if mode in ("nosq", "c_sub"):
                        continue
                    # square (+eps) in place; optionally offload the last
                    # dve_sq 512-col slices (v-tile first) to DVE as plain
                    # tensor_mul (eps dropped there: |2*eps*u| <= 2.4e-5
                    # per element, ~1e-6 on the loss)
                    if fuse_uv:
                        # one ACT + one DVE instruction across u|v; dve_sq
                        # counts TOTAL 512-slices here
                        dv = min(dve_sq, 4 * cn // GROUP)
                        a_hi = 4 * cn - dv * GROUP
                        nc.scalar.activation(
                            uvt[:, :a_hi], uvt[:, :a_hi], act.Square,
                            bias=eps_t[:],
                        )
                        if dv:
                            nc.vector.tensor_mul(
                                uvt[:, a_hi:], uvt[:, a_hi:], uvt[:, a_hi:]
                            )
                    elif fine:
                        # squares per (tensor, half), h-outer; dve_sq
                        # slices come off the end of each tensor's h1
                        dv = min(dve_sq, cn // GROUP)
                        a_hi = 2 * cn - dv * GROUP
                        for t in (ut, vt):
                            nc.scalar.activation(
                                t[:, :cn], t[:, :cn], act.Square,
                                bias=eps_t[:],
                            )
                        for t in (ut, vt):
                            if a_hi > cn:
                                nc.scalar.activation(
                                    t[:, cn:a_hi], t[:, cn:a_hi], act.Square,
                                    bias=eps_t[:],
                                )
                            if dv:
                                nc.vector.tensor_mul(
                                    t[:, a_hi:], t[:, a_hi:], t[:, a_hi:]
                                )
                    else:
                        ncols = 2 * cn // GROUP  # 8 slices of 512 per tile
                        dv = min(dve_sq, ncols)
                        for t in (ut, vt):
                            a_hi = (ncols - dv) * GROUP
                            if a_hi:
                                nc.scalar.activation(
                                    t[:, :a_hi], t[:, :a_hi], act.Square,
                                    bias=eps_t[:],
                                )
                            if dv:
                                nc.vector.tensor_mul(
                                    t[:, a_hi:], t[:, a_hi:], t[:, a_hi:]
                                )
                    if mode in ("nored", "c_sq"):
                        continue
                    # dsplit: h-outer so all h0 matmuls issue before any
                    # h1 work gates on the second half-DMA
                    if dsplit:
                        mm_order = [(h, gl) for h in range(2)
                                    for gl in range(gpc)]
                    else:
                        mm_order = [(h, gl) for gl in range(gpc)
                                    for h in range(2)]
                    first = mm_order[0]
                    last = mm_order[-1]
                    for h, gl in mm_order:
                        g = k * gpc + gl
                        sg = sta[:, g * 32 : (g + 1) * 32]
                        for t, bank in ((ut, dp), (vt, dn)):
                            mv = t[:, h * cn + gl * GROUP :
                                   h * cn + gl * GROUP + GROUP]
                            nc.tensor.matmul(
                                bank[:],
                                sg,
                                mv,
                                start=(k == 0 and (h, gl) == first),
                                stop=(k == n_chunks - 1 and (h, gl) == last),
                            )
                if mode in ("dma", "nosq", "nored", "c_sub", "c_sq"):
                    return
                dps = outp.tile([N_GROUPS, GROUP], F32, tag="dps")
                dns = outp.tile([N_GROUPS, GROUP], F32, tag="dns")
                nc.scalar.activation(dps[:], dp[:], act.Sqrt)
                nc.scalar.activation(dns[:], dn[:], act.Sqrt)
                hng = outp.tile([N_GROUPS, GROUP], F32, tag="hng")
                nc.vector.tensor_sub(hng[:], dps[:], dns[:])
                rel = outp.tile([N_GROUPS, GROUP], F32, tag="rel")
                hsum = outp.tile([N_GROUPS, 1], F32, tag="hsum")
                nc.scalar.activation(
                    rel[:], hng[:], act.Relu, bias=mar_t[:], accum_out=hsum[:]
                )
                nc.sync.dma_start(out[:], hsum[:])

            if loop and repeat > 1:
                assert repeat % unroll == 0
                kw = {}
                if stagger:
                    kw["staggered_reset"] = True
                if hints:
                    kw["hint_engines"] = (mybir.EngineType.PE,)
                with tc.For_i(0, repeat // unroll, 1, **kw):
                    for _ in range(unroll):
                        rep_body()
            else:
                for _ in range(repeat):
                    rep_body()
    nc.compile()
    return nc


def repack_fused(
    x, y, z, chunk_n: int = CHUNK_N, in_dt: str = "bf16",
    dsplit: bool = False,
) -> np.ndarray:
    """Downcast to bf16 (or TRN fp8 e4m3) and repack transposed+interleaved.

    Returns [N_CORES * n_chunks, P, 6*chunk_n]; axis 0 shards evenly
    across cores. Partition p of chunk k holds, for each tensor t in
    (x, y, z) and D-half h, the chunk's rows at feature d = h*128 + p.
    Span order per partition: tensor-major x0|x1|y0|y1|z0|z1, or h-major
    x0|y0|z0|x1|y1|z1 when dsplit (so each half-chunk DMA carries a
    complete (x, y, z) D-half)."""
    np_dt = ml_dtypes.bfloat16 if in_dt == "bf16" else ml_dtypes.float8_e4m3
    n_chunks = SHARD // chunk_n
    parts = []
    for a in (x, y, z):
        ab = np.ascontiguousarray(a, dtype=np.float32).astype(np_dt)
        # [core, chunk, row, d] -> [core, chunk, h, p, row]
        ar = ab.reshape(N_CORES, n_chunks, chunk_n, 2, P)
        parts.append(ar.transpose(0, 1, 3, 4, 2))
    if dsplit:
        order = [(0, 0), (1, 0), (2, 0), (0, 1), (1, 1), (2, 1)]
    else:
        order = [(0, 0), (0, 1), (1, 0), (1, 1), (2, 0), (2, 1)]
    # -> [core, chunk, p, 6, row]
    s = np.stack(
        [parts[t][:, :, h] for t, h in order], axis=2
    ).transpose(0, 1, 3, 2, 4)
    return np.ascontiguousarray(
        s.reshape(N_CORES * n_chunks, P, 6 * chunk_n)
    )


def _run(nc: bass.Bass, x, y, z, chunk_n: int = CHUNK_N, in_dt: str = BEST_IN_DT):
    packed = repack_fused(x, y, z, chunk_n, in_dt,
                          dsplit=BEST_KW.get("dsplit", False))
    n_chunks = SHARD // chunk_n
    in_maps = [
        {"xyz": np.ascontiguousarray(packed[i * n_chunks : (i + 1) * n_chunks])}
        for i in range(N_CORES)
    ]
    return bass_utils.run_bass_kernel_spmd(
        nc, in_maps, core_ids=list(range(N_CORES))
    )


_NC_CACHE = None


def kernel(x: np.ndarray, y: np.ndarray, z: np.ndarray) -> np.ndarray:
    global _NC_CACHE
    x = np.asarray(x, dtype=np.float32)
    y = np.asarray(y, dtype=np.float32)
    z = np.asarray(z, dtype=np.float32)
    if _NC_CACHE is None:
        kw = {k: v for k, v in BEST_KW.items() if k not in ("hints", "unroll")}
        _NC_CACHE = build_nc(1, **kw)
    res = _run(_NC_CACHE, x, y, z, in_dt=BEST_IN_DT)
    total = np.float64(0.0)
    for r in res.results:
        total += r["out"].astype(np.float64).sum()
    return np.float32(total)
